# revision 88
# baseline (speedup 1.0000x reference)
"""Trainium2 Bass kernel for sliding-window (±64) multi-head attention.

Reference computation (seq=4096, hidden=768, 12 heads x 64, RoPE, window 128):
    qkv = qkv_weight @ x ; q,k = rope(q,k) ; scores = q^T k / 8 + band_mask
    attn = softmax(scores) @ v ; out = out_weight @ attn

Sharding: sequence-parallel over 8 cores. Core c owns queries
[512c, 512c+512) and computes K/V over the haloed span [512c-64, 512c+576)
(zero-padded at the sequence edges; padding is killed by the band mask).
Each core runs an identical Bass program on different data; the full output
is reassembled on host by concatenation (no collectives needed).

v3 design (transposed scores): scores are computed directly in [k, q]
orientation (lhsT = K, rhs = Q against zero-padded per-head query blocks),
so the softmax matrix never needs a PE transpose and the PV matmul consumes
the exp output straight from SBUF. The additive band mask (-1e4 out of
window, exp underflows to exact 0) rides the score accumulation group as
identity matmuls. Softmax denominators come from ones-selector matmuls on
the PE ([2, q] per head pair); normalization is deferred to one per-head-
pair multiply against a PE-broadcast row of reciprocals, fused into the
PSUM->SBUF copy of the attention output. All SBUF elementwise work is bf16
(2x DVE rate). Walrus/HW constraints honored: no partition-sliced memsets
(constants arrive by DMA), Pool never reads PSUM, elementwise ops read at
most one PSUM operand, and every PSUM region gets its own start-write.
DMAs are batched per tensor, split across both HWDGE queues, and ordered
by first use (X + first Q/K slices + cos/sin first, W_out last).
"""

import os
import sys

import numpy as np

for _p in ("/opt/trn_rl_repo",):
    if _p not in sys.path and os.path.isdir(_p):
        sys.path.insert(0, _p)

import ml_dtypes

import concourse.bass as bass
import concourse.bacc as bacc
import concourse.tile as tile
from concourse import mybir
from concourse.bass_utils import run_bass_kernel_spmd

F32 = mybir.dt.float32
F32R = mybir.dt.float32r
BF16 = mybir.dt.bfloat16
F8 = mybir.dt.float8e4

# fp8 (e4m3) DoubleRow projections: 4x PE throughput in exchange for ~4%
# weight/activation quantization noise. Weights are pre-scaled by powers of
# two to stay in e4m3's normal range; the scales are folded back out via the
# exp() scale immediate (Q*K) and the SELT reciprocal-broadcast constant (V).
FP8_QK = False
FP8_V = False
SQ = 8   # Wq scale exponent
SK = 6   # Wk scale exponent
SV = 6   # Wv scale exponent

N_CORES = 8
SEQ = 4096
S_CORE = SEQ // N_CORES  # 512 queries per core
HALO = 64                # window // 2
SPAN = S_CORE + 2 * HALO  # 640 keys per core
HID = 768
NH = 12
DH = 64
NCH = HID // 128         # 6 contraction chunks
NHP = NH // 2            # 6 head pairs
NQB = S_CORE // 128      # 4 query blocks per core
NSC = SPAN // 128        # 5 key chunks per core
KSPAN = 256              # key span per query block

_BUILD_CACHE = {}


def _build(add_mask: bool):
    """Build + compile the per-core Bass program (shared by all 8 cores)."""
    nc = bacc.Bacc("TRN2", target_bir_lowering=False, debug=False, num_devices=N_CORES)

    if not (FP8_QK and FP8_V):
        xin = nc.dram_tensor("xin", [128, NCH * SPAN], BF16, kind="ExternalInput")
    if FP8_QK or FP8_V:
        xin8 = nc.dram_tensor("xin8", [128, 3 * 2 * SPAN], F8, kind="ExternalInput")
    if FP8_QK:
        wqt = nc.dram_tensor("wqt", [128, NHP * NCH * 128], F8, kind="ExternalInput")
        wkt = nc.dram_tensor("wkt", [128, NHP * NCH * 128], F8, kind="ExternalInput")
    else:
        wqt = nc.dram_tensor("wqt", [128, NHP * NCH * 128], BF16, kind="ExternalInput")
        wkt = nc.dram_tensor("wkt", [128, NHP * NCH * 128], BF16, kind="ExternalInput")
    wvt = nc.dram_tensor(
        "wvt", [128, NCH * HID], F8 if FP8_V else BF16, kind="ExternalInput"
    )
    wot = nc.dram_tensor("wot", [128, NCH * HID], BF16, kind="ExternalInput")
    # cos | sin, bf16
    cossin = nc.dram_tensor("cossin", [128, 2 * SPAN], BF16, kind="ExternalInput")
    # perms(128) | diag(128) | band-mask^T slots (3*256) | EHS selectors (4)
    pmmt = nc.dram_tensor(
        "pmmt", [128, 256 + 3 * 256 + 4], BF16, kind="ExternalInput"
    )
    selt = nc.dram_tensor("selt", [2, 128], F32R, kind="ExternalInput")
    qzero = nc.dram_tensor("qzero", [64, 3 * S_CORE], BF16, kind="ExternalInput")
    if add_mask:
        maskft = nc.dram_tensor(
            "maskft", [128, NQB * 512], BF16, kind="ExternalInput"
        )
    out_d = nc.dram_tensor("out", [128, NCH * S_CORE], BF16, kind="ExternalOutput")

    mult = mybir.AluOpType.mult
    addop = mybir.AluOpType.add
    exp = mybir.ActivationFunctionType.Exp

    with tile.TileContext(nc) as tc:
        from contextlib import ExitStack

        with ExitStack() as ctx, nc.allow_low_precision(reason="bf16 softmax"):
            const = ctx.enter_context(tc.tile_pool(name="const", bufs=1))
            sb = ctx.enter_context(tc.tile_pool(name="sb", bufs=1))
            tmp = ctx.enter_context(tc.tile_pool(name="tmp", bufs=4))
            praw_p = ctx.enter_context(tc.tile_pool(name="praw_p", bufs=8))
            rr_p = ctx.enter_context(tc.tile_pool(name="rr_p", bufs=2))
            outp = ctx.enter_context(tc.tile_pool(name="outp", bufs=6))
            ps_proj = ctx.enter_context(
                tc.tile_pool(name="ps_proj", bufs=3, space="PSUM")
            )
            ps_sT = ctx.enter_context(tc.tile_pool(name="ps_sT", bufs=3, space="PSUM"))
            ps_o = ctx.enter_context(tc.tile_pool(name="ps_o", bufs=1, space="PSUM"))
            ps_cs = ctx.enter_context(tc.tile_pool(name="ps_cs", bufs=1, space="PSUM"))

            # small constants arrive by DMA (walrus rejects partition-sliced
            # memsets); SELT carries the 2^-SV V-descale on its ones.
            SELT = const.tile([2, 128], F32R, tag="SELT")

            # ---- input DMAs, split over both HWDGE queues (SP + Activation)
            # and sliced so the first-needed pieces land earliest: cos/sin
            # (tiny), X, Q/K weights for head pairs 0-3 (unblocks proj+rope),
            # then the band mask, V weights, the rest, W_out last. ----
            if FP8_QK or FP8_V:
                X8 = const.tile([128, 3, 2, SPAN], F8, tag="X8")
                nc.sync.dma_start(
                    out=X8[:], in_=xin8.ap().rearrange("p (j t s) -> p j t s", j=3, t=2)
                )
            wdt = F8 if FP8_QK else BF16
            WQ = const.tile([128, NHP * NCH * 128], wdt, tag="WQ")
            WK = const.tile([128, NHP * NCH * 128], wdt, tag="WK")
            HPW = 3 * NCH * 128  # three head pairs of weight columns
            if not (FP8_QK and FP8_V):
                X = const.tile([128, NCH * SPAN], BF16, tag="X")
                XH = NCH * SPAN // 2
                nc.sync.dma_start(out=X[:, 0:XH], in_=xin.ap()[:, 0:XH])
                nc.scalar.dma_start(out=WQ[:, 0:HPW], in_=wqt.ap()[:, 0:HPW])
                nc.sync.dma_start(out=X[:, XH:], in_=xin.ap()[:, XH:])
                Xc = [X[:, k * SPAN : (k + 1) * SPAN] for k in range(NCH)]
            else:
                nc.scalar.dma_start(out=WQ[:, 0:HPW], in_=wqt.ap()[:, 0:HPW])
            nc.scalar.dma_start(out=WK[:, 0:HPW], in_=wkt.ap()[:, 0:HPW])
            CS = const.tile([128, 2 * SPAN], BF16, tag="CS")
            nc.sync.dma_start(out=CS[:], in_=cossin.ap())
            COS = CS[:, 0:SPAN]
            SINP = CS[:, SPAN : 2 * SPAN]
            # warm the ACT exp table before the pipeline needs it
            dmy = const.tile([128, 1], F32, tag="dmy")
            nc.scalar.activation(dmy[:], CS[:, 0:1], exp)
            PM = const.tile([128, 256 + 3 * 256 + 4], BF16, tag="PM")
            nc.scalar.dma_start(out=PM[:], in_=pmmt.ap())
            PERMS = PM[:, 0:128]
            DIAG = PM[:, 128:256]
            MTS = [PM[:, 256 + s * 256 : 256 + (s + 1) * 256] for s in range(3)]
            EHS = PM[:, 256 + 768 : 256 + 768 + 4]
            # Qz zero-halves for head pairs 0-2 land first (gate the first
            # score matmuls); pairs 3-5 and the V/W tails follow.
            Qz = sb.tile([128, NHP * 2 * S_CORE], BF16, tag="Qz")
            Qzv = Qz[:].rearrange("p (hp h s) -> p hp h s", hp=NHP, h=2)
            qzv = qzero.ap().rearrange("p (hp s) -> p hp s", hp=3)
            nc.sync.dma_start(out=Qzv[64:128, 0:3, 0, :], in_=qzv[:])
            nc.sync.dma_start(out=Qzv[0:64, 0:3, 1, :], in_=qzv[:])
            WVT = const.tile([128, NCH * HID], F8 if FP8_V else BF16, tag="WVT")
            WVH = NCH * HID // 2
            nc.sync.dma_start(out=WVT[:, 0:WVH], in_=wvt.ap()[:, 0:WVH])
            nc.sync.dma_start(out=SELT[:], in_=selt.ap())
            nc.scalar.dma_start(out=WQ[:, HPW:], in_=wqt.ap()[:, HPW:])
            nc.sync.dma_start(out=WVT[:, WVH:], in_=wvt.ap()[:, WVH:])
            nc.scalar.dma_start(out=WK[:, HPW:], in_=wkt.ap()[:, HPW:])
            nc.sync.dma_start(out=Qzv[64:128, 3:6, 0, :], in_=qzv[:])
            nc.sync.dma_start(out=Qzv[0:64, 3:6, 1, :], in_=qzv[:])
            if add_mask:
                MFT = const.tile([128, NQB * 512], BF16, tag="MFT")
                nc.sync.dma_start(out=MFT[:], in_=maskft.ap())
            WOT = sb.tile([128, NCH * HID], BF16, tag="WOT")
            nc.scalar.dma_start(out=WOT[:], in_=wot.ap())

            # persistent intermediates
            Ks = sb.tile([128, NHP * SPAN], BF16, tag="Ks")     # [2hd, (hp, s)]
            VT = sb.tile([128, NSC * HID], BF16, tag="VT")      # [s, (chunk, hd)]
            AT = sb.tile([128, NCH * S_CORE], BF16, tag="AT")   # [c, (hp, s)]

            # ---- V^T projection: VT[s, hd] per (half, 128-key chunk) ----
            def vt_part(i):
                hf, sc = i // NSC, i % NSC
                w = HID // 2  # 384
                vp = ps_proj.tile([128, S_CORE], F32, tag="proj")
                for k in range(NCH):
                    nc.tensor.matmul(
                        vp[:, :w],
                        Xc[k][:, sc * 128 : (sc + 1) * 128],
                        WVT[:, (hf * NCH + k) * w : (hf * NCH + k + 1) * w],
                        start=(k == 0),
                        stop=(k == NCH - 1),
                    )
                nc.vector.tensor_copy(
                    VT[:, sc * HID + hf * w : sc * HID + (hf + 1) * w],
                    vp[:, :w],
                )

            def rope_q(hp, src_ps):
                w = S_CORE
                qsb = tmp.tile([128, S_CORE], BF16, tag="ropeq")
                nc.scalar.copy(qsb[:, :w], src_ps)
                a = tmp.tile([128, S_CORE], BF16, tag="ropea")
                nc.vector.tensor_tensor(
                    a[:, :w], qsb[:, :w], COS[:, HALO : HALO + w], op=mult
                )
                b = tmp.tile([128, S_CORE], BF16, tag="ropeb")
                nc.vector.tensor_tensor(
                    b[:, :w], qsb[:, :w], SINP[:, HALO : HALO + w], op=mult
                )
                # PERM output reuses the projection PSUM tile (halves proj
                # pool pressure; the scheduler orders it after a/b read qp)
                nc.tensor.matmul(
                    src_ps, PERMS, b[:, :w], start=True, stop=True,
                    skip_group_check=True,
                )
                for h in range(2):
                    nc.vector.tensor_tensor(
                        Qz[64 * h : 64 * h + 64,
                           (hp * 2 + h) * S_CORE : (hp * 2 + h + 1) * S_CORE],
                        src_ps[64 * h : 64 * h + 64, :],
                        a[64 * h : 64 * h + 64, :w],
                        op=addop,
                    )

            def rope(dst, src_ps, cos_ap, sin_ap, w, add_on_pool):
                # dst = src*cos + rot(src)*sin ; rot via PE permutation matmul
                # (sin[d] == sin[d^32] within each 64-half, so rot(q*sin) ==
                # rot(q)*sin and the permutation can run on q*sin directly).
                # Pool cannot read PSUM, so the projection is evacuated to
                # bf16 SBUF once (ACT); the multiplies then run at the DVE/
                # Pool bf16 rates and the final add reads one PSUM operand.
                qsb = tmp.tile([128, S_CORE], BF16, tag="ropeq")
                nc.scalar.copy(qsb[:, :w], src_ps)
                a = tmp.tile([128, S_CORE], BF16, tag="ropea")
                nc.gpsimd.tensor_tensor(a[:, :w], qsb[:, :w], cos_ap, op=mult)
                b = tmp.tile([128, S_CORE], BF16, tag="ropeb")
                nc.vector.tensor_tensor(b[:, :w], qsb[:, :w], sin_ap, op=mult)
                nc.tensor.matmul(
                    src_ps, PERMS, b[:, :w], start=True, stop=True,
                    skip_group_check=True,
                )
                nc.vector.tensor_tensor(dst, src_ps, a[:, :w], op=addop)

            # ---- per head pair: project Q,K then rope into Qs/Ks (bf16) ----
            def proj_hp(hp):
                qp = ps_proj.tile([128, S_CORE], F32, tag="proj")
                if FP8_QK:
                    wq5 = WQ[:].rearrange(
                        "p (hp j t m) -> p hp j t m", hp=NHP, j=3, t=2
                    )
                    for j in range(3):
                        nc.tensor.matmul(
                            qp[:],
                            wq5[:, hp, j],
                            X8[:, j, :, HALO : HALO + S_CORE],
                            start=(j == 0),
                            stop=(j == 2),
                            perf_mode=mybir.MatmulPerfMode.DoubleRow,
                        )
                else:
                    for k in range(NCH):
                        nc.tensor.matmul(
                            qp[:],
                            WQ[:, (hp * NCH + k) * 128 : (hp * NCH + k + 1) * 128],
                            Xc[k][:, HALO : HALO + S_CORE],
                            start=(k == 0),
                            stop=(k == NCH - 1),
                        )
                rope_q(hp, qp[:])
                for half in range(2):
                    w = SPAN // 2  # 320
                    kp = ps_proj.tile([128, S_CORE], F32, tag="proj")
                    if FP8_QK:
                        wk5 = WK[:].rearrange(
                            "p (hp j t m) -> p hp j t m", hp=NHP, j=3, t=2
                        )
                        for j in range(3):
                            nc.tensor.matmul(
                                kp[:, :w],
                                wk5[:, hp, j],
                                X8[:, j, :, half * w : (half + 1) * w],
                                start=(j == 0),
                                stop=(j == 2),
                                perf_mode=mybir.MatmulPerfMode.DoubleRow,
                            )
                    else:
                        for k in range(NCH):
                            nc.tensor.matmul(
                                kp[:, :w],
                                WK[:, (hp * NCH + k) * 128 : (hp * NCH + k + 1) * 128],
                                Xc[k][:, half * w : (half + 1) * w],
                                start=(k == 0),
                                stop=(k == NCH - 1),
                            )
                    rope(
                        Ks[:, hp * SPAN + half * w : hp * SPAN + (half + 1) * w],
                        kp[:, :w],
                        COS[:, half * w : (half + 1) * w],
                        SINP[:, half * w : (half + 1) * w],
                        w,
                        add_on_pool=True,
                    )

            # ---- attention stages per unit (hp, qb) ----
            # S0: scores^T [k, (h, kc, q)] in one PSUM bank; the matmuls
            # form one accumulation group (start zeroes the bank once) and
            # write disjoint column ranges, h0 contracting rows 0:64 and h1
            # rows 64:128 of Qs/Ks. The additive band mask (-1e4 outside the
            # window / invalid keys, so exp underflows to exactly 0) rides
            # the same group as one identity matmul, replacing a separate
            # element-wise mask stage.
            def stage_scores(st):
                hp, qb = st["hp"], st["qb"]
                sT = ps_sT.tile([128, 512], F32, tag="sT", name=f"sT_{hp}_{qb}")
                slot = MTS[0 if qb == 0 else (2 if qb == NQB - 1 else 1)]
                nmm = 5 if add_mask else 4
                i = 0
                for h in range(2):
                    for kc in range(2):
                        i += 1
                        nc.tensor.matmul(
                            sT[:, h * 256 + kc * 128 : h * 256 + kc * 128 + 128],
                            Ks[:, hp * SPAN + qb * 128 + kc * 128 :
                               hp * SPAN + qb * 128 + kc * 128 + 128],
                            Qz[:, (hp * 2 + h) * S_CORE + qb * 128 :
                               (hp * 2 + h) * S_CORE + (qb + 1) * 128],
                            start=(i == 1),
                            stop=(i == nmm),
                            skip_group_check=True,
                        )
                if add_mask:
                    i += 1
                    nc.tensor.matmul(
                        sT[:],
                        DIAG,
                        MFT[:, qb * 512 : (qb + 1) * 512],
                        start=False,
                        stop=True,
                        skip_group_check=True,
                    )
                st["sT"] = sT

            exp_scale = float(2.0 ** (-(SQ + SK))) if FP8_QK else 1.0

            def stage_exp(st):
                praw = praw_p.tile([128, 512], BF16, tag="praw")
                nc.scalar.activation(praw[:], st["sT"][:], exp, scale=exp_scale)
                st["praw"] = praw
                del st["sT"]

            def stage_mask(st):
                qb = st["qb"]
                slot = MTS[0 if qb == 0 else (2 if qb == NQB - 1 else 1)]
                phat = praw_p.tile([128, 512], BF16, tag="phat")
                for h in range(2):
                    eng = nc.vector if h == 0 else nc.gpsimd
                    eng.tensor_tensor(
                        phat[:, h * 256 : (h + 1) * 256],
                        st["praw"][:, h * 256 : (h + 1) * 256],
                        slot, op=mult,
                    )
                st["phat"] = phat
                del st["praw"]

            # S3: PV accumulation into o2[(h,d), q] (one bank per head pair,
            # one long accumulation group over qb/h/kc) + column sums via
            # ones-selector matmuls into cs[2, q]; after qb=3: reciprocal,
            # PE row-broadcast of 1/sum, and the normalize-multiply fused
            # into the PSUM->SBUF evacuation of o2.
            def stage_pv(st):
                hp, qb = st["hp"], st["qb"]
                if qb == 0:
                    o2s[hp] = ps_o.tile([128, S_CORE], F32, tag="o",
                                        name=f"o2_{hp}")
                    css[hp] = ps_cs.tile([2, S_CORE], F32, tag="cs",
                                         name=f"cs_{hp}")
                o2 = o2s[hp]
                cs = css[hp]
                phat = st["phat"]
                # each PSUM output region needs its own start-write (start
                # only zeroes the cells the instruction itself touches)
                for h in range(2):
                    hg = hp * 2 + h
                    for kc in range(2):
                        nc.tensor.matmul(
                            o2[64 * h : 64 * h + 64, qb * 128 : (qb + 1) * 128],
                            VT[:, (qb + kc) * HID + hg * 64 :
                               (qb + kc) * HID + hg * 64 + 64],
                            phat[:, h * 256 + kc * 128 : h * 256 + kc * 128 + 128],
                            start=(kc == 0),
                            stop=(kc == 1),
                            tile_position=(0, 64 * h),
                            skip_group_check=True,
                        )
                for h in range(2):
                    for kc in range(2):
                        nc.tensor.matmul(
                            cs[0:2, qb * 128 : (qb + 1) * 128],
                            EHS[:, 2 * h : 2 * h + 2],
                            phat[:, h * 256 + kc * 128 : h * 256 + kc * 128 + 128],
                            start=(h == 0 and kc == 0),
                            stop=(h == 1 and kc == 1),
                            skip_group_check=True,
                        )
                del st["phat"]

            # deferred per-head-pair tail (reciprocal -> PE row-broadcast ->
            # normalize-multiply), emitted two pipeline steps after the head
            # pair's last PV so the in-order PE queue never waits on the DVE
            # reciprocal.
            def hp_tail(hp):
                cs = css.pop(hp)
                o2 = o2s.pop(hp)
                with tc.high_priority(offset=80):
                    ob = rr_p.tile([128, S_CORE], BF16, tag="ob")
                    rr = rr_p.tile([2, S_CORE], F32R, tag="rr")
                    R = ps_proj.tile([128, S_CORE], F32, tag="proj",
                                     name=f"R_{hp}")
                    nc.scalar.copy(ob[:], o2[:])
                    nc.vector.reciprocal(rr[:], cs[:])
                    nc.tensor.matmul(R[:], SELT[:], rr[:], start=True,
                                     stop=True)
                    nc.vector.tensor_tensor(
                        AT[:, hp * S_CORE : (hp + 1) * S_CORE], R[:],
                        ob[:], op=mult,
                    )

            o2s = {}
            css = {}

            PO1 = sb.tile([128, NCH * S_CORE], BF16, tag="PO1")

            def outproj_part1():
                for oc in range(NCH):
                    ops = ps_proj.tile([128, S_CORE], F32, tag="proj")
                    for k in range(5):
                        nc.tensor.matmul(
                            ops[:],
                            WOT[:, k * HID + oc * 128 : k * HID + (oc + 1) * 128],
                            AT[:, k * S_CORE : (k + 1) * S_CORE],
                            start=(k == 0),
                            stop=(k == 4),
                        )
                    nc.vector.tensor_copy(
                        PO1[:, oc * S_CORE : (oc + 1) * S_CORE], ops[:]
                    )

            proj_hp(0)
            proj_hp(1)
            proj_hp(2)

            units = [
                {"hp": hp, "qb": qb} for hp in range(NHP) for qb in range(NQB)
            ]
            stages = [stage_scores, stage_exp, stage_mask, stage_pv]
            NU = len(units)
            ND = len(stages)
            for step in range(NU + ND + 2):
                # PE filler spread through the pipeline: remaining V^T units
                # early, remaining projections ahead of each head pair's
                # first unit.
                if step < 10:
                    vt_part(step)
                if step < NU:
                    hp, qb = units[step]["hp"], units[step]["qb"]
                    if qb == 0 and 1 <= hp <= 3:
                        proj_hp(hp + 2)
                for k in range(ND - 1, -1, -1):
                    idx = step - k
                    if 0 <= idx < NU:
                        stages[k](units[idx])
                # head-pair tail two steps after its last PV was emitted
                tidx = step - ND - 1
                if 0 <= tidx < NU and units[tidx]["qb"] == NQB - 1:
                    hp_tail(units[tidx]["hp"])


            outproj_part1()

            # ---- output projection tail: the final head-pair-5 chunk, the
            # PO1 combine, and the PSUM evacuation. The combine alternates
            # between DVE (scalar_tensor_tensor) and PE-add + ACT-copy so
            # the six chunks drain the tail on two engine paths in parallel.
            # PO1 combine via PE needs PO1 in a matmul-able dtype: it is
            # F32, so route it through the DIAG identity (bf16 x f32 is not
            # allowed) - instead those chunks use an ACT copy after a PE
            # accumulation of DIAG @ PO1bf (bf16 copy of PO1).
            for oc in range(NCH):
                pool2 = ps_proj if oc % 2 == 0 else ps_sT
                ops = pool2.tile([128, S_CORE], F32,
                                 tag="proj" if oc % 2 == 0 else "sT")
                for k in range(5, NCH):
                    nc.tensor.matmul(
                        ops[:],
                        WOT[:, k * HID + oc * 128 : k * HID + (oc + 1) * 128],
                        AT[:, k * S_CORE : (k + 1) * S_CORE],
                        start=(k == 5),
                        stop=(k == NCH - 1 and oc % 2 == 1),
                        skip_group_check=True,
                    )
                ot = outp.tile([128, S_CORE], BF16, tag="ot")
                if oc % 2 == 1:
                    nc.vector.scalar_tensor_tensor(
                        out=ot[:], in0=ops[:], scalar=1.0,
                        in1=PO1[:, oc * S_CORE : (oc + 1) * S_CORE],
                        op0=mult, op1=addop,
                    )
                else:
                    # fold the PO1 add onto the PE (identity matmul into the
                    # open accumulation) and evacuate via ACT, so the tail
                    # drains on two engine paths in parallel
                    nc.tensor.matmul(
                        ops[:], DIAG,
                        PO1[:, oc * S_CORE : (oc + 1) * S_CORE],
                        start=False, stop=True, skip_group_check=True,
                    )
                    nc.scalar.copy(ot[:], ops[:])
                eng = nc.sync if oc % 2 == 0 else nc.scalar
                eng.dma_start(
                    out=out_d.ap()[:, oc * S_CORE : (oc + 1) * S_CORE], in_=ot[:]
                )

    nc.compile()
    return nc


def get_program(add_mask: bool):
    key = add_mask
    if key not in _BUILD_CACHE:
        _BUILD_CACHE[key] = _build(add_mask)
    return _BUILD_CACHE[key]


def _pack_chunked(a, nch, w):
    """[nch*128, w] row-major -> [128, nch*w] with chunk-major free dim."""
    return np.ascontiguousarray(
        a.reshape(nch, 128, w).transpose(1, 0, 2).reshape(128, nch * w)
    )


def prep_core_inputs(core, xs, pos, am, qkv_weight, out_weight, add_mask):
    """Build the per-core input map (numpy) for one core."""
    start = S_CORE * core - HALO
    idx = np.arange(start, start + SPAN)
    valid = (idx >= 0) & (idx < SEQ)

    Xs = np.zeros((HID, SPAN), np.float32)
    Xs[:, valid] = xs[:, idx[valid]]

    pspan = np.zeros((SPAN,), np.float32)
    pspan[valid] = pos[idx[valid]]
    invf = (
        1.0 / (10000.0 ** (np.arange(0, DH, 2, dtype=np.float32) / np.float32(DH)))
    ).astype(np.float32)
    f = pspan[None, :] * invf[:, None]  # [32, SPAN]
    cos32 = np.cos(f).astype(np.float32)
    sin32 = np.sin(f).astype(np.float32)
    COS = np.tile(cos32, (4, 1))
    SINP = np.tile(sin32, (4, 1))
    cossin = np.concatenate([COS, SINP], axis=1).astype(ml_dtypes.bfloat16)

    # signed rotate-half permutation: (PERMS.T @ q)[d] = rot_half(q)[d]
    di = np.arange(128)
    lo = (di % 64) < 32
    src = np.where(lo, di + 32, di - 32)
    sgn = np.where(lo, -1.0, 1.0).astype(np.float32)
    PERMS = np.zeros((128, 128), np.float32)
    PERMS[src, di] = sgn

    # transposed additive band mask slots [128 k, (kc, q)]: 0 in-window,
    # -10000 outside/invalid so exp(score + mask) underflows to exactly 0.
    # Slot 0 serves qb0, slot 1 qb1/qb2, slot 2 qb3 (identical patterns
    # except at the global sequence edges on cores 0 and 7).
    p = np.arange(128)
    slots = np.zeros((128, 3, 2, 128), np.float32)
    mft = np.zeros((128, NQB, 2, 2, 128), np.float32)
    sscale = np.float32(2.0 ** (SQ + SK)) if FP8_QK else np.float32(1.0)
    for si, qb in ((0, 0), (1, 1), (2, NQB - 1)):
        qg = S_CORE * core + 128 * qb + np.arange(128)  # [q]
        for kc in range(2):
            kg = S_CORE * core + 128 * qb - HALO + kc * 128 + p  # [k]
            kvalid = (kg >= 0) & (kg < SEQ)
            band = (np.abs(kg[:, None] - qg[None, :]) <= HALO) & kvalid[:, None]
            slots[:, si, kc, :] = band.astype(np.float32)
    if add_mask:
        for qb in range(NQB):
            qg = S_CORE * core + 128 * qb + np.arange(128)
            for kc in range(2):
                kg = S_CORE * core + 128 * qb - HALO + kc * 128 + p
                kvalid = (kg >= 0) & (kg < SEQ)
                amt = np.zeros((128, 128), np.float32)
                amt[kvalid, :] = am[np.ix_(qg, kg[kvalid])].T * sscale
                mft[:, qb, 0, kc, :] = amt
                mft[:, qb, 1, kc, :] = amt

    ehs = np.zeros((128, 4), np.float32)
    ehs[:, 0] = 1.0
    ehs[:, 3] = 1.0
    pmmt = np.concatenate(
        [PERMS, np.eye(128, dtype=np.float32), slots.reshape(128, 3 * 256), ehs],
        axis=1,
    ).astype(ml_dtypes.bfloat16)
    selv = np.float32(2.0 ** (-SV)) if FP8_V else np.float32(1.0)
    seltd = np.zeros((2, 128), np.float32)
    seltd[0, 0:64] = selv
    seltd[1, 64:128] = selv

    wq = qkv_weight[0:HID] * np.float32(DH**-0.5)
    wk = qkv_weight[HID : 2 * HID]
    wv = qkv_weight[2 * HID : 3 * HID]

    F8NP = ml_dtypes.float8_e4m3

    def packw(w, dtype=ml_dtypes.bfloat16):
        return _pack_chunked(
            np.ascontiguousarray(w.T.astype(dtype)), NCH, HID
        )

    def packw_hp(w):
        # [c, o] -> [128, (hp, cchunk, 128)] so per-head-pair slices are
        # contiguous in the free dimension
        wt = np.ascontiguousarray(w.T.astype(ml_dtypes.bfloat16))  # [768c, 768o]
        a = wt.reshape(NCH, 128, NHP, 128)  # (cchunk, p, hp, n)
        return np.ascontiguousarray(
            a.transpose(1, 2, 0, 3).reshape(128, NHP * NCH * 128)
        )

    def packw_hp8(w, scale):
        # [o, c] -> [128 p, (hp, j, t, m)]: value = w[hp*128+m, 128*(2j+t)+p]
        ws = (w * np.float32(scale)).astype(F8NP).astype(np.float32)
        a = ws.reshape(NHP, 128, 3, 2, 128)  # (hp, m, j, t, p)
        return np.ascontiguousarray(
            a.transpose(4, 0, 2, 3, 1).reshape(128, NHP * NCH * 128)
        ).astype(F8NP)

    def pack_x8(xs_span):
        # [c, s] -> [128 p, (j, t, s)]
        a = xs_span.reshape(3, 2, 128, SPAN)  # (j, t, p, s)
        return np.ascontiguousarray(
            a.transpose(2, 0, 1, 3).reshape(128, 6 * SPAN)
        ).astype(F8NP)

    def packwv8(w, scale):
        # [o, c] -> [128 p, (j, t, o)]: value = w[o, 128*(2j+t)+p]
        ws = (w * np.float32(scale)).astype(F8NP).astype(np.float32)
        a = ws.reshape(HID, 3, 2, 128)  # (o, j, t, p)
        return np.ascontiguousarray(
            a.transpose(3, 1, 2, 0).reshape(128, NCH * HID)
        ).astype(F8NP)

    in_map = {
        "wot": packw(out_weight),
        "cossin": cossin,
        "pmmt": pmmt,
        "selt": seltd,
        "qzero": np.zeros((64, 3 * S_CORE), dtype=ml_dtypes.bfloat16),
    }
    if FP8_QK or FP8_V:
        in_map["xin8"] = pack_x8(Xs)
    if not (FP8_QK and FP8_V):
        in_map["xin"] = _pack_chunked(Xs.astype(ml_dtypes.bfloat16), NCH, SPAN)
    if FP8_QK:
        in_map["wqt"] = packw_hp8(wq, 2.0 ** SQ)
        in_map["wkt"] = packw_hp8(wk, 2.0 ** SK)
    else:
        in_map["wqt"] = packw_hp(wq)
        in_map["wkt"] = packw_hp(wk)
    # hf-major wvt: [128 p, (hf, chunk, 384)]
    wvt_t = np.ascontiguousarray(wv.T.astype(ml_dtypes.bfloat16))  # [768c, 768o]
    a = wvt_t.reshape(NCH, 128, 2, 384)  # (chunk, p, hf, j)
    in_map["wvt"] = np.ascontiguousarray(
        a.transpose(1, 2, 0, 3).reshape(128, NCH * HID)
    )
    if add_mask:
        in_map["maskft"] = np.ascontiguousarray(
            mft.reshape(128, NQB * 512)
        ).astype(ml_dtypes.bfloat16)
    return in_map


def prep_all_inputs(x, position_ids, attention_mask, qkv_weight, out_weight):
    xs = np.asarray(x, dtype=np.float32)[0, :, 0, :]  # [768, 4096]
    pos = np.asarray(position_ids)[0].astype(np.float32)
    am = np.asarray(attention_mask, dtype=np.float32)[0, 0]
    qkv_w = np.asarray(qkv_weight, dtype=np.float32)
    out_w = np.asarray(out_weight, dtype=np.float32)
    add_mask = bool(np.any(am))
    in_maps = [
        prep_core_inputs(c, xs, pos, am, qkv_w, out_w, add_mask)
        for c in range(N_CORES)
    ]
    return in_maps, add_mask


def assemble_output(results):
    cols = []
    for c in range(N_CORES):
        o = np.asarray(results[c]["out"])  # [128, 6*512]
        cols.append(o.reshape(128, NCH, S_CORE).transpose(1, 0, 2).reshape(HID, S_CORE))
    full = np.concatenate(cols, axis=1)  # [768, 4096]
    return np.ascontiguousarray(full.reshape(1, HID, 1, SEQ), dtype=np.float32)


def kernel(**inputs):
    in_maps, add_mask = prep_all_inputs(
        inputs["x"],
        inputs["position_ids"],
        inputs["attention_mask"],
        inputs["qkv_weight"],
        inputs["out_weight"],
    )
    nc = get_program(add_mask)
    res = run_bass_kernel_spmd(nc, in_maps, core_ids=list(range(N_CORES)))
    return assemble_output(res.results)
